# revision 1
# baseline (speedup 1.0000x reference)
"""GAT layer kernel for 8 Trainium2 NeuronCores — v2.

Same edge-parallel / dense-rank strategy as v1, plus:
  - fp16 table rows padded to 128 halfs (256B); gather elements are row
    PAIRS (512B) — the DMA sweet spot (sub-512B descriptors pay a 2x
    read-modify-write penalty, so 512B is the minimum-cost element).
  - sigma node reordering (n -> interleaved row) makes phase-B table
    writes contiguous 512B runs per partition.
  - the per-edge accumulation runs on the PE as identity-lhsT matmuls
    accumulating into 49 per-block PSUM regions (7 banks x 7 blocks),
    freeing the DVE of the [128,sl,65] multiply-add chain.
  - leaky-relu/exp/PSUM-copies run on the scalar (ACT) engine.
  - gathers on 2 alternating SWDGE queues with a 36KB/queue descriptor
    carveout: two 1024-descriptor gathers now fit in-flight per queue
    (the old 32KB carveout was one descriptor short of 2x1024, which
    serialized every gather against the previous one's drain).
"""
import os
import sys
import numpy as np

_ABL = set(os.environ.get("KGAT_ABLATE", "").split(","))
_REP = int(os.environ.get("KGAT_REPEAT", "1"))

try:
    import concourse.bacc as bacc
except ImportError:
    sys.path.insert(0, "/opt/trn_rl_repo")
    import concourse.bacc as bacc
import concourse.tile as tile
import concourse.mybir as mybir
from concourse import bass_utils
from concourse.masks import make_identity

C = 8
GQ_TOK = int(os.environ.get("KGAT_GQ", "1024"))
NQ = int(os.environ.get("KGAT_NQ", "2"))
TROW = 128            # fp16 per table row (256B)
PAIR = 2 * TROW       # fp16 per gather element (512B)

F32 = mybir.dt.float32
F16 = mybir.dt.float16
I16 = mybir.dt.int16


def _sig(n):
    """Node -> table row: within each 256-group interleave halves so that
    phase B (two 128-node matmul tiles per group) writes row pairs
    (2p, 2p+1) contiguously from partition p."""
    return (n // 256) * 256 + 2 * (n % 128) + (n % 256) // 128


def _wrap16(a):
    w = np.ascontiguousarray(a.reshape(-1, 16).T)
    return np.tile(w, (8, 1))


def _prep(edge_index, edge_weight, N):
    NL = N // C
    src = np.asarray(edge_index[0], dtype=np.int64)
    dst = np.asarray(edge_index[1], dtype=np.int64)
    w = np.asarray(edge_weight, dtype=np.float32)

    cores = []
    max_cnt = np.zeros(0, np.int64)
    for c in range(C):
        m = (dst >= c * NL) & (dst < (c + 1) * NL)
        s_c = src[m]
        d_c = dst[m] - c * NL
        w_c = w[m]
        deg = np.bincount(d_c, minlength=NL)
        perm = np.argsort(-deg, kind="stable")
        order = np.argsort(d_c, kind="stable")
        starts = np.zeros(NL + 1, np.int64)
        starts[1:] = np.cumsum(deg)
        maxdeg = int(deg.max()) if deg.size else 0
        hist = np.bincount(deg, minlength=maxdeg + 2)
        cnt = NL - np.cumsum(hist)[:maxdeg + 1]
        cnt = cnt[cnt > 0]
        cores.append(dict(s=s_c, w=w_c, perm=perm, order=order,
                          starts=starts, cnt=cnt))
        if len(cnt) > len(max_cnt):
            mc = np.zeros(len(cnt), np.int64)
            mc[:len(max_cnt)] = max_cnt
            max_cnt = mc
        max_cnt[:len(cnt)] = np.maximum(max_cnt[:len(cnt)], cnt)

    sched = []
    rank_base = []
    tot = 0
    for r in range(len(max_cnt)):
        c128 = int(-(-max_cnt[r] // 128) * 128)
        rank_base.append(tot)
        p = 0
        while p < c128:
            K = min(GQ_TOK, c128 - p)
            sched.append((K, K // 128, p // 128))
            p += K
        tot += c128

    NLP = -(-NL // 128) * 128
    per_core = []
    for c in range(C):
        cc = cores[c]
        perm, order, starts, cnt = cc["perm"], cc["order"], cc["starts"], cc["cnt"]
        pair = np.zeros(tot, np.int16)
        par = np.zeros(tot, np.float16)
        wt = np.zeros(tot, np.float16)
        mask = np.zeros(tot, np.float16)
        for r in range(len(max_cnt)):
            n = int(cnt[r]) if r < len(cnt) else 0
            if n == 0:
                continue
            o = rank_base[r]
            eid = order[starts[perm[:n]] + r]
            sg = _sig(cc["s"][eid])
            pair[o:o + n] = (sg >> 1).astype(np.int16)
            par[o:o + n] = (sg & 1).astype(np.float16)
            wt[o:o + n] = cc["w"][eid].astype(np.float16)
            mask[o:o + n] = 1.0

        sidx_cols, meta_cols = [], []
        off = 0
        for (K, slots, boff) in sched:
            sidx_cols.append(_wrap16(pair[off:off + K]))
            def tl(a):
                return np.ascontiguousarray(a[off:off + K].reshape(slots, 128).T)
            meta_cols.append(np.stack([tl(par), tl(wt), tl(mask)],
                                      axis=2).reshape(128, slots * 3))
            off += K
        sidx = np.concatenate(sidx_cols, axis=1)
        meta = np.ascontiguousarray(np.concatenate(meta_cols, axis=1),
                                    dtype=np.float16)

        gnode = np.zeros(NLP, np.int64)
        gnode[:NL] = _sig(c * NL + perm)
        pgidx_cols = []
        p = 0
        while p < NLP:
            K = min(GQ_TOK, NLP - p)
            pgidx_cols.append(_wrap16((gnode[p:p + K] >> 1).astype(np.int16)))
            p += K
        pgidx = np.concatenate(pgidx_cols, axis=1)
        ppar = np.ascontiguousarray(
            (gnode & 1).astype(np.float16).reshape(NLP // 128, 128).T)

        per_core.append(dict(sidx=sidx, meta=meta, pgidx=pgidx, ppar=ppar))

    return sched, per_core, [cores[c]["perm"] for c in range(C)], tot, NLP


_BUILD_CACHE = {}


def _build(N, F, O, sched, tot, NLP):
    key = (N, F, O, tuple(sched), tot, NLP)
    if key in _BUILD_CACHE:
        return _BUILD_CACHE[key]
    NB = NLP // 128
    assert NB <= 56, NB            # 7 blocks x 8 PSUM banks
    TOT16 = tot // 16
    TOT128 = tot // 128
    NPAD = -(-N // 256) * 256

    # first/last chunk touching each position block (PSUM start/stop flags)
    first_t = {}
    last_t = {}
    for j, (K, sl, boff) in enumerate(sched):
        for s in range(sl):
            b = boff + s
            if b not in first_t:
                first_t[b] = j
            last_t[b] = j

    nc = bacc.Bacc("TRN2", target_bir_lowering=False,
                   dynamic_dma_scratch_size=int(os.environ.get("KGAT_SCR", "36864")),
                   num_swdge_queues=NQ)
    x_t = nc.dram_tensor("x", [F, N], F32, kind="ExternalInput")
    w_t = nc.dram_tensor("W", [F, O], F32, kind="ExternalInput")
    a_t = nc.dram_tensor("a", [2 * O], F32, kind="ExternalInput")
    sidx_t = nc.dram_tensor("sidx", [128, TOT16], I16, kind="ExternalInput")
    meta_t = nc.dram_tensor("meta", [128, TOT128 * 3], F16, kind="ExternalInput")
    pgidx_t = nc.dram_tensor("pgidx", [128, NLP // 16], I16, kind="ExternalInput")
    ppar_t = nc.dram_tensor("ppar", [128, NB], F16, kind="ExternalInput")
    out_t = nc.dram_tensor("out", [NLP, O], F32, kind="ExternalOutput")

    with tile.TileContext(nc) as tc:
        with (
            tc.tile_pool(name="persist", bufs=1) as pp,
            tc.tile_pool(name="dram", bufs=1, space="DRAM") as dp,
            tc.tile_pool(name="work", bufs=4) as wp,
            tc.tile_pool(name="gpool", bufs=3) as gp,
            tc.tile_pool(name="pgpool", bufs=1) as pgp,
            tc.tile_pool(name="streams", bufs=1) as sp,
            tc.tile_pool(name="final", bufs=1) as fp,
        ):
            table = dp.tile([NPAD, TROW], F16)
            tpair = table[:].rearrange("(p two) r -> p (two r)", two=2)

            # ---- phase A: waug = [W | W@a1 | W@a2] (f32) ----
            ident = pp.tile([128, 128], F32)
            make_identity(nc, ident[:])
            idf = pp.tile([128, 128], F16)
            nc.vector.tensor_copy(idf[:], ident[:])
            ws = pp.tile([128, O], F32)
            nc.sync.dma_start(ws[:], w_t[:])
            a1 = pp.tile([O, 1], F32)
            a2 = pp.tile([O, 1], F32)
            nc.sync.dma_start(a1[:], a_t[:O, None])
            nc.sync.dma_start(a2[:], a_t[O:, None])
            with tc.tile_pool(name="psA", bufs=1, space="PSUM") as psa:
                wtp = psa.tile([O, 128], F32, space="PSUM")
                nc.tensor.transpose(out=wtp[:], in_=ws[:], identity=ident[:])
                wts = pp.tile([O, 128], F32)
                nc.vector.tensor_copy(wts[:], wtp[:])
                vab = psa.tile([128, 2], F32, space="PSUM")
                nc.tensor.matmul(out=vab[:, 0:1], lhsT=wts[:], rhs=a1[:],
                                 start=True, stop=True)
                nc.tensor.matmul(out=vab[:, 1:2], lhsT=wts[:], rhs=a2[:],
                                 start=True, stop=True)
                waug = pp.tile([128, O + 2], F32)
                nc.vector.tensor_copy(waug[:, :O], ws[:])
                nc.vector.tensor_copy(waug[:, O:], vab[:])

            for rep in range(_REP):
              # ---- phase B: table rows [h | s_src | s_dst | 0...] fp16 ----
              XB = 8
              nbt = NPAD // (XB * 128)
              with tc.tile_pool(name=f"psB{rep}", bufs=2, space="PSUM") as psb:
                for t in range(0 if "phaseb" in _ABL else nbt):
                    n0 = t * XB * 128
                    nn = min(XB * 128, N - n0)          # real nodes this batch
                    ng = -(-nn // 128)
                    xts = wp.tile([F, XB * 128], F32, tag="xts")
                    nc.sync.dma_start(xts[:, :nn], x_t[:, n0:n0 + nn])
                    hs = wp.tile([128, XB // 2, 2 * TROW], F16, tag="hs")
                    nc.vector.memset(hs[:], 0.0)
                    for g in range(ng):
                        nr = min(128, nn - g * 128)
                        hp = psb.tile([128, O + 2], F32, space="PSUM", tag="hp")
                        nc.tensor.matmul(
                            out=hp[:nr],
                            lhsT=xts[:, g * 128:g * 128 + nr],
                            rhs=waug[:], start=True, stop=True)
                        co = (g % 2) * TROW
                        nc.scalar.activation(
                            hs[:nr, g // 2, co:co + O + 2], hp[:nr],
                            mybir.ActivationFunctionType.Copy)
                    nc.sync.dma_start(
                        table[n0:n0 + XB * 128, :].rearrange(
                            "(g p two) r -> p g (two r)", p=128, two=2),
                        hs[:])

              # ---- stream preload ----
              sidxS = sp.tile([128, TOT16], I16, tag="sidx")
              nc.sync.dma_start(sidxS[:], sidx_t[:])
              metaS = sp.tile([128, TOT128 * 3], F16, tag="meta")
              nc.sync.dma_start(metaS[:], meta_t[:])
              pgS = sp.tile([128, NLP // 16], I16, tag="pg")
              nc.sync.dma_start(pgS[:], pgidx_t[:])
              pparS = sp.tile([128, NB], F16, tag="ppar")
              nc.sync.dma_start(pparS[:], ppar_t[:])

              # ---- phase B2: s_dst per node position (permuted order) ----
              sdp = sp.tile([128, NB], F16, tag="sdp")
              p = 0
              qn = 0
              while p < NLP:
                  K = min(GQ_TOK, NLP - p)
                  sl = K // 128
                  b0 = p // 128
                  pg = pgp.tile([128, GQ_TOK // 128, PAIR], F16, tag="pgt")
                  nc.gpsimd.dma_gather(
                      out_ap=pg[:, :sl, :], in_ap=tpair,
                      idxs_ap=pgS[:, p // 16:(p + K) // 16],
                      num_idxs=K, num_idxs_reg=K, elem_size=PAIR,
                      queue_num=qn % NQ, single_packet=False)
                  qn += 1
                  lo = pg[:, :sl, O + 1]
                  hi = pg[:, :sl, TROW + O + 1]
                  tmp = wp.tile([128, GQ_TOK // 128], F16, tag="sdtmp")
                  nc.vector.tensor_sub(tmp[:, :sl], hi, lo)
                  nc.vector.tensor_mul(tmp[:, :sl], tmp[:, :sl],
                                       pparS[:, b0:b0 + sl])
                  nc.vector.tensor_add(sdp[:, b0:b0 + sl], lo, tmp[:, :sl])
                  p += K

              # ---- edge phase: gather + alpha + PE accumulate into PSUM ----
              chunks = []
              o16 = 0
              o128 = 0
              for (K, sl, boff) in sched:
                  chunks.append((K, sl, boff, o16, o128))
                  o16 += K // 16
                  o128 += sl
              use_pe = os.environ.get("KGAT_PE", "1") == "1"
              if use_pe:
                  waves = {}
                  for ch in chunks:
                      assert ch[2] % 8 == 0 and ch[1] <= 8
                      waves.setdefault(ch[2] // 8, []).append(ch)
                  order = [ch for w in sorted(waves) for ch in waves[w]]
              else:
                  order = chunks
              with tc.tile_pool(name=f"psE{rep}", bufs=1, space="PSUM") as pse:
                if not use_pe:
                    dacc = fp.tile([128, NB, O + 1], F16, tag="dacc")
                apw = None
                wcur = -1
                sacc = fp.tile([128, NB, O + 1], F32, tag="sacc")
                if use_pe:
                    nc.vector.memset(sacc[:], 0.0)
                for j, (K, sl, boff, o16, o128) in enumerate(order):
                    if use_pe and boff // 8 != wcur:
                        wcur = boff // 8
                        wl = waves[wcur]
                        apw = pse.tile([128, 8, 512], F32, space="PSUM",
                                       tag="apw")
                    gt = gp.tile([128, GQ_TOK // 128, PAIR], F16, tag="gt")
                    if "gather" not in _ABL:
                        nc.gpsimd.dma_gather(
                            out_ap=gt[:, :sl, :], in_ap=tpair,
                            idxs_ap=sidxS[:, o16:o16 + K // 16],
                            num_idxs=K, num_idxs_reg=K, elem_size=PAIR,
                            queue_num=j % NQ, single_packet=False)
                    mt3 = metaS[:, o128 * 3:(o128 + sl) * 3].rearrange(
                        "p (s k) -> p s k", k=3)
                    par = mt3[:, :, 0]
                    wgt = mt3[:, :, 1]
                    msk = mt3[:, :, 2]
                    if "dve" in _ABL:
                        continue

                    sel = wp.tile([128, GQ_TOK // 128, O + 1], F16, tag="sel")
                    parb = par[:, :, None].to_broadcast([128, sl, O + 1])
                    if os.environ.get("KGAT_SEL3", "1") == "1":
                        nc.vector.tensor_sub(sel[:, :sl, :],
                                             gt[:, :sl, TROW:TROW + O + 1],
                                             gt[:, :sl, :O + 1])
                        nc.vector.tensor_mul(sel[:, :sl, :], sel[:, :sl, :],
                                             parb)
                        nc.vector.tensor_add(sel[:, :sl, :], sel[:, :sl, :],
                                             gt[:, :sl, :O + 1])
                    else:
                        nc.scalar.activation(sel[:, :sl, :],
                                             gt[:, :sl, :O + 1],
                                             mybir.ActivationFunctionType.Copy)
                        nc.vector.copy_predicated(sel[:, :sl, :], parb,
                                                  gt[:, :sl, TROW:TROW + O + 1])

                    e = wp.tile([128, GQ_TOK // 128], F16, tag="e")
                    nc.vector.tensor_add(e[:, :sl], sel[:, :sl, O],
                                         sdp[:, boff:boff + sl])
                    lk = wp.tile([128, GQ_TOK // 128], F16, tag="lk")
                    if os.environ.get("KGAT_LRELU", "0") != "1":
                        nc.vector.tensor_scalar(out=lk[:, :sl], in0=e[:, :sl],
                                                scalar1=0.0, scalar2=0.2,
                                                op0=mybir.AluOpType.min,
                                                op1=mybir.AluOpType.mult)
                        nc.vector.tensor_scalar(out=e[:, :sl], in0=e[:, :sl],
                                                scalar1=0.0, scalar2=None,
                                                op0=mybir.AluOpType.max)
                        nc.vector.tensor_add(lk[:, :sl], lk[:, :sl], e[:, :sl])
                    else:
                        nc.scalar.activation(lk[:, :sl], e[:, :sl],
                                             mybir.ActivationFunctionType.Lrelu,
                                             alpha=0.2)
                    nc.vector.tensor_mul(lk[:, :sl], lk[:, :sl], wgt)
                    ax = wp.tile([128, GQ_TOK // 128], F16, tag="ax")
                    nc.scalar.activation(ax[:, :sl], lk[:, :sl],
                                         mybir.ActivationFunctionType.Exp)
                    nc.vector.tensor_mul(ax[:, :sl], ax[:, :sl], msk)

                    nc.vector.memset(sel[:, :sl, O], 1.0)
                    nc.vector.tensor_mul(
                        sel[:, :sl, :], sel[:, :sl, :],
                        ax[:, :sl, None].to_broadcast([128, sl, O + 1]))
                    if not use_pe:
                        if j == 0:
                            nc.vector.memset(dacc[:], 0.0)
                        nc.vector.tensor_add(dacc[:, boff:boff + sl, :],
                                             dacc[:, boff:boff + sl, :],
                                             sel[:, :sl, :])
                    else:
                        i = wl.index((K, sl, boff, o16, o128))
                        for s in range(sl):
                            last_i = max(ii for ii, ch in enumerate(wl)
                                         if ch[1] > s)
                            nc.tensor.matmul(
                                out=apw[:, s, :O + 1],
                                lhsT=idf[:], rhs=sel[:, s, :],
                                start=(i == 0), stop=(i == last_i),
                                skip_group_check=True)
                            if i == last_i:
                                nc.scalar.activation(
                                    sacc[:, boff + s, :], apw[:, s, :O + 1],
                                    mybir.ActivationFunctionType.Copy)

                # ---- final: out = elu(S / (alpha_sum + 1e-8)) ----
                if "dve" in _ABL:
                    nc.vector.memset(sacc[:], 0.0)
                elif not use_pe:
                    nc.vector.tensor_copy(sacc[:], dacc[:])

              rc = fp.tile([128, NB], F32, tag="rc")
              nc.vector.tensor_scalar(out=rc[:], in0=sacc[:, :, O],
                                      scalar1=1e-8, scalar2=None,
                                      op0=mybir.AluOpType.add)
              nc.vector.reciprocal(rc[:], rc[:])
              ov = fp.tile([128, NB, O], F32, tag="ov")
              nc.vector.tensor_mul(ov[:], sacc[:, :, :O],
                                   rc[:, :, None].to_broadcast([128, NB, O]))
              neg = fp.tile([128, NB, O], F32, tag="neg")
              nc.vector.tensor_scalar(out=neg[:], in0=ov[:], scalar1=0.0,
                                      scalar2=None, op0=mybir.AluOpType.min)
              nc.scalar.activation(neg[:], neg[:],
                                   mybir.ActivationFunctionType.Exp)
              nc.vector.tensor_scalar(out=ov[:], in0=ov[:], scalar1=0.0,
                                      scalar2=-1.0, op0=mybir.AluOpType.max,
                                      op1=mybir.AluOpType.add)
              nc.vector.tensor_add(ov[:], ov[:], neg[:])
              nc.sync.dma_start(
                  out_t[:].rearrange("(b p) f -> p b f", p=128), ov[:])

    nc.compile()
    _BUILD_CACHE[key] = nc
    return nc


def kernel(x, edge_index, edge_weight, W, a):
    x = np.asarray(x, dtype=np.float32)
    xT = np.ascontiguousarray(x.T)
    W = np.ascontiguousarray(np.asarray(W, dtype=np.float32))
    a = np.ascontiguousarray(np.asarray(a, dtype=np.float32))
    N, F = x.shape
    O = W.shape[1]
    NL = N // C

    sched, per_core, perms, tot, NLP = _prep(edge_index, edge_weight, N)
    nc = _build(N, F, O, sched, tot, NLP)

    in_maps = []
    for c in range(C):
        pc = per_core[c]
        in_maps.append({
            "x": xT, "W": W, "a": a,
            "sidx": pc["sidx"], "meta": pc["meta"],
            "pgidx": pc["pgidx"], "ppar": pc["ppar"],
        })
    res = bass_utils.run_bass_kernel_spmd(nc, in_maps, core_ids=list(range(C)))

    out = np.empty((N, O), np.float32)
    for c in range(C):
        op = res.results[c]["out"]
        out[c * NL + perms[c]] = op[:NL]
    return out



# revision 5
# speedup vs baseline: 1.5791x; 1.5791x over previous
"""GAT layer kernel for 8 Trainium2 NeuronCores — v3.

Edge-parallel dense-rank strategy (dst-sharded) as v2, restructured for
engine balance (v2 was DVE- and phase-B-serialization-bound):

  - pair gathers unchanged (512B elements, GQ=1024, NQ=2 — measured
    optimal at 0.87 ns/idx on HW; 256B elements run at half the rate).
  - alpha math batched over superchunks of 8 chunks (64 slots wide) on
    [128, 64] tiles: ~10 medium DVE ops per 8192 edges instead of ~10
    small ops per 1024 edges. lrelu fused via scalar_tensor_tensor.
  - the hi/lo pair select is folded into the alpha weighting:
    contribution = axl*lo + axh*hi with axl = ax*(1-par), axh = ax*par.
    Two big DVE muls per chunk; no 3-op select chain.
  - accumulation into PSUM-resident [128, 7, 7*65] f32 (7 windows x 7
    blocks x 65 cols = all 49 dst blocks live in PSUM), via identity-lhsT
    matmuls over maximal block-runs; alpha_sum rides along as rhs col 64.
  - phase B: fp16 x input (half the read bytes), 4-block PSUM tiles,
    2 ACT copies per 4 blocks, XB=16 batches, dedicated pools.
  - s_dst per position from a host-permuted fp16 x copy via 49 one-col
    matmuls (replaces v2's phase-B2 table gathers).
"""
import os
import sys
import numpy as np

_REP = int(os.environ.get("KGAT_REPEAT", "1"))

try:
    import concourse.bacc as bacc
except ImportError:
    sys.path.insert(0, "/opt/trn_rl_repo")
    import concourse.bacc as bacc
import concourse.tile as tile
import concourse.mybir as mybir
from concourse import bass_utils
from concourse.masks import make_identity

C = 8
GQ = int(os.environ.get("KGAT_GQ", "1024"))
NQ = int(os.environ.get("KGAT_NQ", "2"))
SCR = int(os.environ.get("KGAT_SCR", "36864"))
SK = int(os.environ.get("KGAT_SK", "8"))      # chunks per superchunk
WIN = 7                                        # blocks per PSUM window
TROW = 128            # fp16 per table row (256B)
PAIR = 2 * TROW       # fp16 per gather element (512B)
XB = 16               # phase-B nodes per batch / 128

F32 = mybir.dt.float32
F16 = mybir.dt.float16
I16 = mybir.dt.int16
ALU = mybir.AluOpType
ACTF = mybir.ActivationFunctionType


def _sig(n):
    """Node -> table row: within each 256-group interleave halves so that
    phase B writes row pairs (2p, 2p+1) contiguously from partition p."""
    return (n // 256) * 256 + 2 * (n % 128) + (n % 256) // 128


def _wrap16(a):
    w = np.ascontiguousarray(a.reshape(-1, 16).T)
    return np.tile(w, (8, 1))


def _prep(edge_index, edge_weight, N):
    NL = N // C
    NB = -(-NL // 128)
    NLP = NB * 128
    src = np.asarray(edge_index[0], dtype=np.int64)
    dst = np.asarray(edge_index[1], dtype=np.int64)
    w = np.asarray(edge_weight, dtype=np.float32)

    cores = []
    max_cnt = np.zeros(0, np.int64)
    for c in range(C):
        m = (dst >= c * NL) & (dst < (c + 1) * NL)
        s_c = src[m]
        d_c = dst[m] - c * NL
        w_c = w[m]
        deg = np.bincount(d_c, minlength=NL)
        perm = np.argsort(-deg, kind="stable")
        order = np.argsort(d_c, kind="stable")
        starts = np.zeros(NL + 1, np.int64)
        starts[1:] = np.cumsum(deg)
        maxdeg = int(deg.max()) if deg.size else 0
        hist = np.bincount(deg, minlength=maxdeg + 2)
        cnt = NL - np.cumsum(hist)[:maxdeg + 1]
        cnt = cnt[cnt > 0]
        cores.append(dict(s=s_c, w=w_c, perm=perm, order=order,
                          starts=starts, cnt=cnt))
        if len(cnt) > len(max_cnt):
            mc = np.zeros(len(cnt), np.int64)
            mc[:len(max_cnt)] = max_cnt
            max_cnt = mc
        max_cnt[:len(cnt)] = np.maximum(max_cnt[:len(cnt)], cnt)

    # rank 0 must cover every position block so its runs span full PSUM
    # windows (start=True resets whole windows)
    max_cnt[0] = NLP

    # unified slot stream: rank-major, block = slot index within rank
    nb_r = [int(-(-int(x) // 128)) for x in max_cnt]
    blocks = []
    rank_slot0 = []
    for r, nb in enumerate(nb_r):
        rank_slot0.append(len(blocks))
        blocks.extend(range(nb))
    S = len(blocks)
    nch = -(-S // 8)
    Spad = nch * 8
    tot = Spad * 128

    # matmul runs: maximal slot runs with consecutive blocks, same window,
    # within one chunk. (o, n, w, b0, first, last)
    raw = []
    i = 0
    while i < S:
        j = i + 1
        while (j < S and j % 8 != 0 and blocks[j] == blocks[j - 1] + 1
               and blocks[j] // WIN == blocks[i] // WIN):
            j += 1
        raw.append([i, j - i, blocks[i] // WIN, blocks[i], False, False])
        i = j
    firstw = {}
    lastw = {}
    for k, run in enumerate(raw):
        if run[2] not in firstw:
            firstw[run[2]] = k
        lastw[run[2]] = k
    for wdx, k in firstw.items():
        raw[k][4] = True
    for wdx, k in lastw.items():
        raw[k][5] = True
    runs_by_chunk = [[] for _ in range(nch)]
    for (o, n, wdx, b0, fi, la) in raw:
        runs_by_chunk[o // 8].append((o % 8, n, wdx, b0, fi, la))

    # per-rank sds copies: (slot offset, num blocks)
    rank_copies = [(rank_slot0[r], nb_r[r]) for r in range(len(nb_r))]

    per_core = []
    for c in range(C):
        cc = cores[c]
        perm, order, starts, cnt = cc["perm"], cc["order"], cc["starts"], cc["cnt"]
        pi = np.zeros(tot, np.int16)
        par = np.zeros(tot, np.float16)
        wt = np.zeros(tot, np.float16)
        msk = np.zeros(tot, np.float16)
        for r in range(len(nb_r)):
            n = int(cnt[r]) if r < len(cnt) else 0
            if n == 0:
                continue
            o = rank_slot0[r] * 128
            eid = order[starts[perm[:n]] + r]
            sg = _sig(cc["s"][eid])
            pi[o:o + n] = (sg >> 1).astype(np.int16)
            par[o:o + n] = (sg & 1).astype(np.float16)
            wt[o:o + n] = cc["w"][eid].astype(np.float16)
            msk[o:o + n] = 1.0
        sidx = _wrap16(pi)

        def tl(a):
            return np.ascontiguousarray(a.reshape(Spad, 128).T)
        per_core.append(dict(sidx=sidx, par=tl(par), wt=tl(wt), msk=tl(msk)))

    perms = [cores[c]["perm"] for c in range(C)]
    sched = dict(S=S, Spad=Spad, nch=nch, tot=tot, NB=NB, NLP=NLP,
                 runs_by_chunk=tuple(
                     tuple(rc) for rc in runs_by_chunk),
                 rank_copies=tuple(rank_copies))
    return sched, per_core, perms


_BUILD_CACHE = {}


def _build(N, F, O, sched):
    key = (N, F, O, sched["Spad"], sched["runs_by_chunk"],
           sched["rank_copies"], _REP)
    if key in _BUILD_CACHE:
        return _BUILD_CACHE[key]
    NB = sched["NB"]
    NLP = sched["NLP"]
    Spad = sched["Spad"]
    nch = sched["nch"]
    tot = sched["tot"]
    runs_by_chunk = sched["runs_by_chunk"]
    rank_copies = sched["rank_copies"]
    NPAD = -(-N // 256) * 256
    assert NB <= WIN * WIN

    nc = bacc.Bacc("TRN2", target_bir_lowering=False,
                   dynamic_dma_scratch_size=SCR, num_swdge_queues=NQ)
    x_t = nc.dram_tensor("x16", [F, N], F16, kind="ExternalInput")
    w_t = nc.dram_tensor("W", [F, O], F32, kind="ExternalInput")
    a_t = nc.dram_tensor("a", [2 * O], F32, kind="ExternalInput")
    xp_t = nc.dram_tensor("xperm", [F, NLP], F16, kind="ExternalInput")
    sidx_t = nc.dram_tensor("sidx", [128, tot // 16], I16, kind="ExternalInput")
    par_t = nc.dram_tensor("par", [128, Spad], F16, kind="ExternalInput")
    wt_t = nc.dram_tensor("wt", [128, Spad], F16, kind="ExternalInput")
    msk_t = nc.dram_tensor("msk", [128, Spad], F16, kind="ExternalInput")
    out_t = nc.dram_tensor("out", [NLP, O], F32, kind="ExternalOutput")

    with tile.TileContext(nc) as tc:
        with (
            tc.tile_pool(name="persist", bufs=1) as pp,
            tc.tile_pool(name="dram", bufs=1, space="DRAM") as dp,
            tc.tile_pool(name="xpool", bufs=3) as xp,
            tc.tile_pool(name="hpool", bufs=3) as hp_pool,
            tc.tile_pool(name="gpool", bufs=8) as gp,
            tc.tile_pool(name="apool", bufs=3) as ap_,
            tc.tile_pool(name="rpool", bufs=2) as rp,
            tc.tile_pool(name="streams", bufs=1) as sp,
            tc.tile_pool(name="final", bufs=1) as fp,
        ):
            table = dp.tile([NPAD, TROW], F16)
            tpair = table[:].rearrange("(p two) r -> p (two r)", two=2)

            # ---- phase A: waug = [W | W@a1] fp16, wa2 fp16 ----
            ident = pp.tile([128, 128], F32)
            make_identity(nc, ident[:])
            idf = pp.tile([128, 128], F16)
            nc.vector.tensor_copy(idf[:], ident[:])
            ws = pp.tile([128, O], F32)
            nc.sync.dma_start(ws[:], w_t[:])
            a1 = pp.tile([O, 1], F32)
            a2 = pp.tile([O, 1], F32)
            nc.sync.dma_start(a1[:], a_t[:O, None])
            nc.sync.dma_start(a2[:], a_t[O:, None])
            waug = pp.tile([128, O + 1], F16)
            wa2f = pp.tile([128, 1], F16)
            with tc.tile_pool(name="psA", bufs=1, space="PSUM") as psa:
                wtp = psa.tile([O, 128], F32, space="PSUM")
                nc.tensor.transpose(out=wtp[:], in_=ws[:], identity=ident[:])
                wts = pp.tile([O, 128], F32)
                nc.vector.tensor_copy(wts[:], wtp[:])
                vab = psa.tile([128, 2], F32, space="PSUM")
                nc.tensor.matmul(out=vab[:, 0:1], lhsT=wts[:], rhs=a1[:],
                                 start=True, stop=True)
                nc.tensor.matmul(out=vab[:, 1:2], lhsT=wts[:], rhs=a2[:],
                                 start=True, stop=True)
                nc.vector.tensor_copy(waug[:, :O], ws[:])
                nc.vector.tensor_copy(waug[:, O:O + 1], vab[:, 0:1])
                nc.vector.tensor_copy(wa2f[:], vab[:, 1:2])

            xpS = sp.tile([128, NLP], F16, tag="xperm")
            nc.sync.dma_start(xpS[:], xp_t[:])

            for rep in range(_REP):
              # ---- streams ----
              sidxS = sp.tile([128, tot // 16], I16, tag="sidx")
              nc.sync.dma_start(sidxS[:], sidx_t[:])
              parS = sp.tile([128, Spad], F16, tag="par")
              nc.sync.dma_start(parS[:], par_t[:])
              wtS = sp.tile([128, Spad], F16, tag="wt")
              nc.sync.dma_start(wtS[:], wt_t[:])
              mskS = sp.tile([128, Spad], F16, tag="msk")
              nc.sync.dma_start(mskS[:], msk_t[:])

              # ---- s_dst per position: 49 one-col matmuls ----
              sdpS = sp.tile([128, NB], F16, tag="sdp")
              sdsS = sp.tile([128, Spad], F16, tag="sds")
              if Spad > sched["S"]:
                  nc.vector.memset(sdsS[:, sched["S"]:Spad], 0.0)
              with tc.tile_pool(name=f"psD{rep}", bufs=1, space="PSUM") as psd:
                pssd = psd.tile([128, NB], F32, space="PSUM")
                for b in range(NB):
                    nc.tensor.matmul(out=pssd[:, b:b + 1],
                                     lhsT=xpS[:, b * 128:(b + 1) * 128],
                                     rhs=wa2f[:], start=True, stop=True)
                nc.scalar.activation(sdpS[:], pssd[:], ACTF.Copy)
              for (so, nb) in rank_copies:
                  nc.scalar.activation(sdsS[:, so:so + nb], sdpS[:, :nb],
                                       ACTF.Copy)

              # ---- phase B: table rows [h | s_src | 0pad] fp16, pair layout ----
              nbt = -(-NPAD // (XB * 128))
              with tc.tile_pool(name=f"psB{rep}", bufs=3, space="PSUM") as psb:
                for t in range(nbt):
                    n0 = t * XB * 128
                    nodes = min(XB * 128, NPAD - n0)       # rows this batch
                    nn = max(0, min(nodes, N - n0))        # real nodes
                    ng = nodes // 128
                    xts = xp.tile([F, XB * 128], F16, tag="xts")
                    if nn < nodes:
                        nc.vector.memset(xts[:, nn:nodes], 0.0)
                    nc.sync.dma_start(xts[:, :nn], x_t[:, n0:n0 + nn])
                    hs = hp_pool.tile([128, XB // 2, PAIR], F16, tag="hs")
                    for q in range(0, ng, 4):
                        qn = min(4, ng - q)
                        hpp = psb.tile([128, 4, O + 1], F32, space="PSUM",
                                       tag="hp")
                        for g in range(q, q + qn):
                            nc.tensor.matmul(
                                out=hpp[:, g - q, :],
                                lhsT=xts[:, g * 128:(g + 1) * 128],
                                rhs=waug[:], start=True, stop=True)
                        # parity 0 blocks -> col 0, parity 1 -> col 128
                        nc.scalar.activation(
                            hs[:, q // 2:q // 2 + qn // 2, 0:O + 1],
                            hpp[:, 0:qn:2, :], ACTF.Copy)
                        nc.scalar.activation(
                            hs[:, q // 2:q // 2 + qn // 2, TROW:TROW + O + 1],
                            hpp[:, 1:qn:2, :], ACTF.Copy)
                    nc.sync.dma_start(
                        table[n0:n0 + nodes, :].rearrange(
                            "(g p two) r -> p g (two r)", p=128, two=2),
                        hs[:, :nodes // 256, :])

              # ---- edge phase ----
              nsc = -(-nch // SK)
              with tc.tile_pool(name=f"psE{rep}", bufs=1, space="PSUM") as pse:
                psacc = pse.tile([128, WIN, 512], F32, space="PSUM", tag="acc")
                for sc in range(nsc):
                    ch0 = sc * SK
                    K = min(SK, nch - ch0)
                    SL = K * 8
                    s0 = ch0 * 8
                    srcS = ap_.tile([128, SK * 8, 2], F16, tag="srcs")
                    gts = []
                    for jj in range(K):
                        j = ch0 + jj
                        gt = gp.tile([128, 8, PAIR], F16, tag="gt")
                        gts.append(gt)
                        nc.gpsimd.dma_gather(
                            out_ap=gt[:, :, :], in_ap=tpair,
                            idxs_ap=sidxS[:, j * GQ // 16:(j + 1) * GQ // 16],
                            num_idxs=GQ, num_idxs_reg=GQ, elem_size=PAIR,
                            queue_num=j % NQ, single_packet=False)
                        nc.scalar.activation(
                            srcS[:, jj * 8:(jj + 1) * 8, :],
                            gt[:, :, O:PAIR:TROW], ACTF.Copy)
                    # batched alpha on [128, SL]
                    lo = srcS[:, :SL, 0]
                    hi = srcS[:, :SL, 1]
                    pr = parS[:, s0:s0 + SL]
                    ed = ap_.tile([128, SK * 8], F16, tag="ed")
                    e = ap_.tile([128, SK * 8], F16, tag="e")
                    ax = ap_.tile([128, SK * 8], F16, tag="ax")
                    axl = ap_.tile([128, SK * 8], F16, tag="axl")
                    axh = ap_.tile([128, SK * 8], F16, tag="axh")
                    nc.vector.tensor_sub(ed[:, :SL], hi, lo)
                    nc.vector.tensor_mul(ed[:, :SL], ed[:, :SL], pr)
                    nc.vector.tensor_add(e[:, :SL], ed[:, :SL], lo)
                    nc.vector.tensor_add(e[:, :SL], e[:, :SL],
                                         sdsS[:, s0:s0 + SL])
                    nc.vector.tensor_mul(e[:, :SL], e[:, :SL],
                                         wtS[:, s0:s0 + SL])
                    nc.vector.scalar_tensor_tensor(
                        out=ax[:, :SL], in0=e[:, :SL], scalar=0.2,
                        in1=e[:, :SL], op0=ALU.mult, op1=ALU.max)
                    nc.scalar.activation(ax[:, :SL], ax[:, :SL], ACTF.Exp)
                    nc.vector.tensor_mul(ax[:, :SL], ax[:, :SL],
                                         mskS[:, s0:s0 + SL])
                    nc.vector.tensor_mul(axh[:, :SL], ax[:, :SL], pr)
                    nc.vector.tensor_sub(axl[:, :SL], ax[:, :SL], axh[:, :SL])

                    rhsA = rp.tile([128, SK * 8, O + 1], F16, tag="rhsA")
                    rhsB = rp.tile([128, SK * 8, O + 1], F16, tag="rhsB")
                    nc.vector.tensor_copy(rhsA[:, :SL, O], axl[:, :SL])
                    nc.vector.tensor_copy(rhsB[:, :SL, O], axh[:, :SL])
                    for jj in range(K):
                        gt = gts[jj]
                        j8 = jj * 8
                        nc.vector.tensor_mul(
                            rhsA[:, j8:j8 + 8, :O], gt[:, :, :O],
                            axl[:, j8:j8 + 8, None].to_broadcast([128, 8, O]))
                        nc.vector.tensor_mul(
                            rhsB[:, j8:j8 + 8, :O], gt[:, :, TROW:TROW + O],
                            axh[:, j8:j8 + 8, None].to_broadcast([128, 8, O]))
                        for (o, n, wdx, b0, fi, la) in runs_by_chunk[ch0 + jj]:
                            oo = jj * 8 + o
                            c0 = (b0 - wdx * WIN) * (O + 1)
                            cn = n * (O + 1)
                            nc.tensor.matmul(
                                out=psacc[:, wdx, c0:c0 + cn],
                                lhsT=idf[:],
                                rhs=rhsA[:, oo:oo + n, :],
                                start=fi, stop=False,
                                skip_group_check=True)
                            nc.tensor.matmul(
                                out=psacc[:, wdx, c0:c0 + cn],
                                lhsT=idf[:],
                                rhs=rhsB[:, oo:oo + n, :],
                                start=False, stop=la,
                                skip_group_check=True)

                # ---- final: out = elu(S / (alpha_sum + 1e-8)) ----
                sacc = fp.tile([128, NB, O + 1], F32, tag="sacc")
                for wdx in range(WIN):
                    nc.scalar.activation(
                        sacc[:, wdx * WIN:(wdx + 1) * WIN, :].rearrange(
                            "p b c -> p (b c)"),
                        psacc[:, wdx, :WIN * (O + 1)], ACTF.Copy)

              rc = fp.tile([128, NB], F32, tag="rc")
              nc.vector.tensor_scalar(out=rc[:], in0=sacc[:, :, O],
                                      scalar1=1e-8, scalar2=None,
                                      op0=ALU.add)
              nc.vector.reciprocal(rc[:], rc[:])
              ov = fp.tile([128, NB, O], F32, tag="ov")
              nc.vector.tensor_mul(ov[:], sacc[:, :, :O],
                                   rc[:, :, None].to_broadcast([128, NB, O]))
              neg = sacc[:, :, :O]
              nc.vector.tensor_scalar(out=neg, in0=ov[:], scalar1=0.0,
                                      scalar2=None, op0=ALU.min)
              nc.scalar.activation(neg, neg, ACTF.Exp)
              nc.vector.tensor_scalar(out=ov[:], in0=ov[:], scalar1=0.0,
                                      scalar2=-1.0, op0=ALU.max,
                                      op1=ALU.add)
              nc.vector.tensor_add(ov[:], ov[:], neg)
              nc.sync.dma_start(
                  out_t[:].rearrange("(b p) f -> p b f", p=128), ov[:])

    nc.compile()
    _BUILD_CACHE[key] = nc
    return nc


def _in_maps(x, W, a, per_core, perms, sched):
    x = np.asarray(x, dtype=np.float32)
    W = np.ascontiguousarray(np.asarray(W, dtype=np.float32))
    a = np.ascontiguousarray(np.asarray(a, dtype=np.float32))
    N, F = x.shape
    NL = N // C
    xT16 = np.ascontiguousarray(x.T.astype(np.float16))
    NLP = sched["NLP"]
    in_maps = []
    for c in range(C):
        pc = per_core[c]
        xperm = np.zeros((F, NLP), np.float16)
        xperm[:, :NL] = xT16[:, c * NL + perms[c]]
        in_maps.append({
            "x16": xT16, "W": W, "a": a, "xperm": xperm,
            "sidx": pc["sidx"], "par": pc["par"], "wt": pc["wt"],
            "msk": pc["msk"],
        })
    return in_maps


def kernel(x, edge_index, edge_weight, W, a):
    x = np.asarray(x, dtype=np.float32)
    W = np.ascontiguousarray(np.asarray(W, dtype=np.float32))
    a = np.ascontiguousarray(np.asarray(a, dtype=np.float32))
    N, F = x.shape
    O = W.shape[1]
    NL = N // C

    sched, per_core, perms = _prep(edge_index, edge_weight, N)
    nc = _build(N, F, O, sched)

    in_maps = _in_maps(x, W, a, per_core, perms, sched)
    res = bass_utils.run_bass_kernel_spmd(nc, in_maps, core_ids=list(range(C)))

    out = np.empty((N, O), np.float32)
    for c in range(C):
        op = res.results[c]["out"]
        out[c * NL + perms[c]] = op[:NL]
    return out


# revision 9
# speedup vs baseline: 3.3705x; 2.1344x over previous
"""GAT layer kernel for 8 Trainium2 NeuronCores — v3.

Edge-parallel dense-rank strategy (dst-sharded) as v2, restructured for
engine balance (v2 was DVE- and phase-B-serialization-bound):

  - pair gathers unchanged (512B elements, GQ=1024, NQ=2 — measured
    optimal at 0.87 ns/idx on HW; 256B elements run at half the rate).
  - alpha math batched over superchunks of 8 chunks (64 slots wide) on
    [128, 64] tiles: ~10 medium DVE ops per 8192 edges instead of ~10
    small ops per 1024 edges. lrelu fused via scalar_tensor_tensor.
  - the hi/lo pair select is folded into the alpha weighting:
    contribution = axl*lo + axh*hi with axl = ax*(1-par), axh = ax*par.
    Two big DVE muls per chunk; no 3-op select chain.
  - accumulation into PSUM-resident [128, 7, 7*65] f32 (7 windows x 7
    blocks x 65 cols = all 49 dst blocks live in PSUM), via identity-lhsT
    matmuls over maximal block-runs; alpha_sum rides along as rhs col 64.
  - phase B: fp16 x input (half the read bytes), 4-block PSUM tiles,
    2 ACT copies per 4 blocks, XB=16 batches, dedicated pools.
  - s_dst per position from a host-permuted fp16 x copy via 49 one-col
    matmuls (replaces v2's phase-B2 table gathers).
"""
import os
import sys
import numpy as np

_REP = int(os.environ.get("KGAT_REPEAT", "1"))
_ABL = set(os.environ.get("KGAT_ABLATE", "").split(","))

try:
    import concourse.bacc as bacc
except ImportError:
    sys.path.insert(0, "/opt/trn_rl_repo")
    import concourse.bacc as bacc
import concourse.tile as tile
import concourse.mybir as mybir
from concourse import bass_utils
from concourse.masks import make_identity

C = 8
GQ = int(os.environ.get("KGAT_GQ", "1024"))
NQ = int(os.environ.get("KGAT_NQ", "2"))
SCR = int(os.environ.get("KGAT_SCR", "36864"))
SK = int(os.environ.get("KGAT_SK", "8"))      # chunks per superchunk
WIN = 7                                        # blocks per PSUM window
TROW = 128            # fp16 per table row (256B)
PAIR = 2 * TROW       # fp16 per gather element (512B)
XB = 16               # phase-B nodes per batch / 128

F32 = mybir.dt.float32
F16 = mybir.dt.float16
I16 = mybir.dt.int16
ALU = mybir.AluOpType
ACTF = mybir.ActivationFunctionType


def _sig(n):
    """Node -> table row: within each 256-group interleave halves so that
    phase B writes row pairs (2p, 2p+1) contiguously from partition p."""
    return (n // 256) * 256 + 2 * (n % 128) + (n % 256) // 128


def _wrap16(a):
    w = np.ascontiguousarray(a.reshape(-1, 16).T)
    return np.tile(w, (8, 1))


def _prep(edge_index, edge_weight, N):
    NL = N // C
    NB = -(-NL // 128)
    NLP = NB * 128
    src = np.asarray(edge_index[0], dtype=np.int64)
    dst = np.asarray(edge_index[1], dtype=np.int64)
    w = np.asarray(edge_weight, dtype=np.float32)

    cores = []
    max_cnt = np.zeros(0, np.int64)
    for c in range(C):
        m = (dst >= c * NL) & (dst < (c + 1) * NL)
        s_c = src[m]
        d_c = dst[m] - c * NL
        w_c = w[m]
        deg = np.bincount(d_c, minlength=NL)
        perm = np.argsort(-deg, kind="stable")
        order = np.argsort(d_c, kind="stable")
        starts = np.zeros(NL + 1, np.int64)
        starts[1:] = np.cumsum(deg)
        maxdeg = int(deg.max()) if deg.size else 0
        hist = np.bincount(deg, minlength=maxdeg + 2)
        cnt = NL - np.cumsum(hist)[:maxdeg + 1]
        cnt = cnt[cnt > 0]
        cores.append(dict(s=s_c, w=w_c, perm=perm, order=order,
                          starts=starts, cnt=cnt))
        if len(cnt) > len(max_cnt):
            mc = np.zeros(len(cnt), np.int64)
            mc[:len(max_cnt)] = max_cnt
            max_cnt = mc
        max_cnt[:len(cnt)] = np.maximum(max_cnt[:len(cnt)], cnt)

    # rank 0 must cover every position block so its runs span full PSUM
    # windows (start=True resets whole windows)
    max_cnt[0] = NLP

    # unified slot stream: rank-major, block = slot index within rank
    nb_r = [int(-(-int(x) // 128)) for x in max_cnt]
    blocks = []
    rank_slot0 = []
    for r, nb in enumerate(nb_r):
        rank_slot0.append(len(blocks))
        blocks.extend(range(nb))
    S = len(blocks)
    nch = -(-S // 8)
    Spad = nch * 8
    tot = Spad * 128

    # matmul runs: maximal slot runs with consecutive blocks, same window,
    # within one chunk. (o, n, w, b0, first, last)
    raw = []
    i = 0
    while i < S:
        j = i + 1
        while (j < S and j % 8 != 0 and blocks[j] == blocks[j - 1] + 1
               and blocks[j] // WIN == blocks[i] // WIN):
            j += 1
        raw.append([i, j - i, blocks[i] // WIN, blocks[i], False, False])
        i = j
    firstw = {}
    lastw = {}
    for k, run in enumerate(raw):
        if run[2] not in firstw:
            firstw[run[2]] = k
        lastw[run[2]] = k
    for wdx, k in firstw.items():
        raw[k][4] = True
    for wdx, k in lastw.items():
        raw[k][5] = True
    runs_by_chunk = [[] for _ in range(nch)]
    for (o, n, wdx, b0, fi, la) in raw:
        runs_by_chunk[o // 8].append((o % 8, n, wdx, b0, fi, la))

    # per-rank sds copies: (slot offset, num blocks)
    rank_copies = [(rank_slot0[r], nb_r[r]) for r in range(len(nb_r))]

    per_core = []
    for c in range(C):
        cc = cores[c]
        perm, order, starts, cnt = cc["perm"], cc["order"], cc["starts"], cc["cnt"]
        pi = np.zeros(tot, np.int16)
        par = np.zeros(tot, np.float16)
        wt = np.zeros(tot, np.float16)
        msk = np.zeros(tot, np.float16)
        for r in range(len(nb_r)):
            n = int(cnt[r]) if r < len(cnt) else 0
            if n == 0:
                continue
            o = rank_slot0[r] * 128
            eid = order[starts[perm[:n]] + r]
            sg = _sig(cc["s"][eid])
            pi[o:o + n] = (sg >> 1).astype(np.int16)
            par[o:o + n] = (sg & 1).astype(np.float16)
            wt[o:o + n] = cc["w"][eid].astype(np.float16)
            msk[o:o + n] = 1.0
        sidx = _wrap16(pi)

        def tl(a):
            return np.ascontiguousarray(a.reshape(Spad, 128).T)
        per_core.append(dict(sidx=sidx, par=tl(par), wt=tl(wt), msk=tl(msk)))

    perms = [cores[c]["perm"] for c in range(C)]
    sched = dict(S=S, Spad=Spad, nch=nch, tot=tot, NB=NB, NLP=NLP,
                 runs_by_chunk=tuple(
                     tuple(rc) for rc in runs_by_chunk),
                 rank_copies=tuple(rank_copies))
    return sched, per_core, perms


_BUILD_CACHE = {}


def _build(N, F, O, sched):
    key = (N, F, O, sched["Spad"], sched["runs_by_chunk"],
           sched["rank_copies"], _REP)
    if key in _BUILD_CACHE:
        return _BUILD_CACHE[key]
    NB = sched["NB"]
    NLP = sched["NLP"]
    Spad = sched["Spad"]
    nch = sched["nch"]
    tot = sched["tot"]
    runs_by_chunk = sched["runs_by_chunk"]
    rank_copies = sched["rank_copies"]
    NPAD = -(-N // 256) * 256
    assert NB <= WIN * WIN

    nc = bacc.Bacc("TRN2", target_bir_lowering=False,
                   dynamic_dma_scratch_size=SCR, num_swdge_queues=NQ)
    x_t = nc.dram_tensor("x16", [F, N], F16, kind="ExternalInput")
    w_t = nc.dram_tensor("W", [F, O], F32, kind="ExternalInput")
    a_t = nc.dram_tensor("a", [2 * O], F32, kind="ExternalInput")
    xp_t = nc.dram_tensor("xperm", [F, NLP], F16, kind="ExternalInput")
    sidx_t = nc.dram_tensor("sidx", [128, tot // 16], I16, kind="ExternalInput")
    par_t = nc.dram_tensor("par", [128, Spad], F16, kind="ExternalInput")
    wt_t = nc.dram_tensor("wt", [128, Spad], F16, kind="ExternalInput")
    msk_t = nc.dram_tensor("msk", [128, Spad], F16, kind="ExternalInput")
    out_t = nc.dram_tensor("out", [NLP, O], F32, kind="ExternalOutput")

    with tile.TileContext(nc) as tc:
        with (
            tc.tile_pool(name="persist", bufs=1) as pp,
            tc.tile_pool(name="dram", bufs=1, space="DRAM") as dp,
            tc.tile_pool(name="xpool", bufs=3) as xp,
            tc.tile_pool(name="hpool", bufs=3) as hp_pool,
            tc.tile_pool(name="gpool", bufs=8) as gp,
            tc.tile_pool(name="apool", bufs=3) as ap_,
            tc.tile_pool(name="rpool", bufs=2) as rp,
            tc.tile_pool(name="streams", bufs=1) as sp,
            tc.tile_pool(name="final", bufs=1) as fp,
        ):
            table = dp.tile([NPAD, TROW], F16)
            tpair = table[:].rearrange("(p two) r -> p (two r)", two=2)

            # ---- phase A: waug = [W | W@a1] fp16, wa2 fp16 ----
            ident = pp.tile([128, 128], F32)
            make_identity(nc, ident[:])
            idf = pp.tile([128, 128], F16)
            nc.vector.tensor_copy(idf[:], ident[:])
            ws = pp.tile([128, O], F32)
            nc.sync.dma_start(ws[:], w_t[:])
            a1 = pp.tile([O, 1], F32)
            a2 = pp.tile([O, 1], F32)
            nc.sync.dma_start(a1[:], a_t[:O, None])
            nc.sync.dma_start(a2[:], a_t[O:, None])
            waug = pp.tile([128, O + 1], F16)
            wa2f = pp.tile([128, 1], F16)
            with tc.tile_pool(name="psA", bufs=1, space="PSUM") as psa:
                wtp = psa.tile([O, 128], F32, space="PSUM")
                nc.tensor.transpose(out=wtp[:], in_=ws[:], identity=ident[:])
                wts = pp.tile([O, 128], F32)
                nc.vector.tensor_copy(wts[:], wtp[:])
                vab = psa.tile([128, 2], F32, space="PSUM")
                nc.tensor.matmul(out=vab[:, 0:1], lhsT=wts[:], rhs=a1[:],
                                 start=True, stop=True)
                nc.tensor.matmul(out=vab[:, 1:2], lhsT=wts[:], rhs=a2[:],
                                 start=True, stop=True)
                nc.vector.tensor_copy(waug[:, :O], ws[:])
                nc.vector.tensor_copy(waug[:, O:O + 1], vab[:, 0:1])
                nc.vector.tensor_copy(wa2f[:], vab[:, 1:2])

            xpS = sp.tile([128, NLP], F16, tag="xperm")
            nc.sync.dma_start(xpS[:], xp_t[:])

            for rep in range(_REP):
              # ---- streams ----
              sidxS = sp.tile([128, tot // 16], I16, tag="sidx")
              nc.sync.dma_start(sidxS[:], sidx_t[:])
              parS = sp.tile([128, Spad], F16, tag="par")
              nc.sync.dma_start(parS[:], par_t[:])
              wtS = sp.tile([128, Spad], F16, tag="wt")
              nc.sync.dma_start(wtS[:], wt_t[:])
              mskS = sp.tile([128, Spad], F16, tag="msk")
              nc.sync.dma_start(mskS[:], msk_t[:])

              # ---- s_dst per position: 49 one-col matmuls ----
              sdpS = sp.tile([128, NB], F16, tag="sdp")
              sdsS = sp.tile([128, Spad], F16, tag="sds")
              if Spad > sched["S"]:
                  nc.vector.memset(sdsS[:, sched["S"]:Spad], 0.0)
              with tc.tile_pool(name=f"psD{rep}", bufs=1, space="PSUM") as psd:
                pssd = psd.tile([128, NB], F32, space="PSUM")
                for b in range(NB):
                    nc.tensor.matmul(out=pssd[:, b:b + 1],
                                     lhsT=xpS[:, b * 128:(b + 1) * 128],
                                     rhs=wa2f[:], start=True, stop=True)
                nc.scalar.activation(sdpS[:], pssd[:], ACTF.Copy)
              for (so, nb) in rank_copies:
                  nc.scalar.activation(sdsS[:, so:so + nb], sdpS[:, :nb],
                                       ACTF.Copy)

              # ---- phase B: table rows [h | s_src | 0pad] fp16, pair layout ----
              nbt = -(-NPAD // (XB * 128))
              if "phaseb" in _ABL:
                  nbt = 0
              with tc.tile_pool(name=f"psB{rep}", bufs=3, space="PSUM") as psb:
                for t in range(nbt):
                    n0 = t * XB * 128
                    nodes = min(XB * 128, NPAD - n0)       # rows this batch
                    nn = max(0, min(nodes, N - n0))        # real nodes
                    ng = nodes // 128
                    xts = xp.tile([F, XB * 128], F16, tag="xts")
                    if nn < nodes:
                        nc.vector.memset(xts[:, nn:nodes], 0.0)
                    nc.sync.dma_start(xts[:, :nn], x_t[:, n0:n0 + nn])
                    hs = hp_pool.tile([128, XB // 2, PAIR], F16, tag="hs")
                    for q in range(0, ng, 4):
                        qn = min(4, ng - q)
                        hpp = psb.tile([128, 4, O + 1], F32, space="PSUM",
                                       tag="hp")
                        for g in range(q, q + qn):
                            nc.tensor.matmul(
                                out=hpp[:, g - q, :],
                                lhsT=xts[:, g * 128:(g + 1) * 128],
                                rhs=waug[:], start=True, stop=True)
                        # parity 0 blocks -> col 0, parity 1 -> col 128
                        nc.scalar.activation(
                            hs[:, q // 2:q // 2 + qn // 2, 0:O + 1],
                            hpp[:, 0:qn:2, :], ACTF.Copy)
                        nc.scalar.activation(
                            hs[:, q // 2:q // 2 + qn // 2, TROW:TROW + O + 1],
                            hpp[:, 1:qn:2, :], ACTF.Copy)
                    nc.sync.dma_start(
                        table[n0:n0 + nodes, :].rearrange(
                            "(g p two) r -> p g (two r)", p=128, two=2),
                        hs[:, :nodes // 256, :])

              # ---- edge phase ----
              nsc = -(-nch // SK)
              with tc.tile_pool(name=f"psE{rep}", bufs=1, space="PSUM") as pse:
                psacc = pse.tile([128, WIN, 512], F32, space="PSUM", tag="acc")
                for sc in range(nsc):
                    ch0 = sc * SK
                    K = min(SK, nch - ch0)
                    SL = K * 8
                    s0 = ch0 * 8
                    srcS = ap_.tile([128, SK * 8, 2], F16, tag="srcs")
                    gts = []
                    for jj in range(K):
                        j = ch0 + jj
                        gt = gp.tile([128, 8, PAIR], F16, tag="gt")
                        gts.append(gt)
                        if "gather" not in _ABL:
                            nc.gpsimd.dma_gather(
                                out_ap=gt[:, :, :], in_ap=tpair,
                                idxs_ap=sidxS[:, j * GQ // 16:(j + 1) * GQ // 16],
                                num_idxs=GQ, num_idxs_reg=GQ, elem_size=PAIR,
                                queue_num=j % NQ, single_packet=False)
                        elif sc == 0 and jj == 0:
                            nc.vector.memset(gt[:], 0.0)
                        nc.scalar.activation(
                            srcS[:, jj * 8:(jj + 1) * 8, :],
                            gt[:, :, O:PAIR:TROW], ACTF.Copy)
                    # batched alpha on [128, SL]
                    lo = srcS[:, :SL, 0]
                    hi = srcS[:, :SL, 1]
                    pr = parS[:, s0:s0 + SL]
                    ed = ap_.tile([128, SK * 8], F16, tag="ed")
                    e = ap_.tile([128, SK * 8], F16, tag="e")
                    ax = ap_.tile([128, SK * 8], F16, tag="ax")
                    axl = ap_.tile([128, SK * 8], F16, tag="axl")
                    axh = ap_.tile([128, SK * 8], F16, tag="axh")
                    nc.vector.tensor_sub(ed[:, :SL], hi, lo)
                    nc.vector.tensor_mul(ed[:, :SL], ed[:, :SL], pr)
                    nc.vector.tensor_add(e[:, :SL], ed[:, :SL], lo)
                    nc.vector.tensor_add(e[:, :SL], e[:, :SL],
                                         sdsS[:, s0:s0 + SL])
                    nc.vector.tensor_mul(e[:, :SL], e[:, :SL],
                                         wtS[:, s0:s0 + SL])
                    nc.vector.scalar_tensor_tensor(
                        out=ax[:, :SL], in0=e[:, :SL], scalar=0.2,
                        in1=e[:, :SL], op0=ALU.mult, op1=ALU.max)
                    nc.scalar.activation(ax[:, :SL], ax[:, :SL], ACTF.Exp)
                    nc.vector.tensor_mul(ax[:, :SL], ax[:, :SL],
                                         mskS[:, s0:s0 + SL])
                    nc.vector.tensor_mul(axh[:, :SL], ax[:, :SL], pr)
                    nc.vector.tensor_sub(axl[:, :SL], ax[:, :SL], axh[:, :SL])

                    rhsA = rp.tile([128, SK * 8, O + 1], F16, tag="rhsA")
                    rhsB = rp.tile([128, SK * 8, O + 1], F16, tag="rhsB")
                    if "mm" in _ABL:
                        if sc == 0:
                            nc.vector.memset(rhsA[:], 0.0)
                            nc.vector.memset(rhsB[:], 0.0)
                        continue
                    nc.vector.tensor_copy(rhsA[:, :SL, O], axl[:, :SL])
                    nc.vector.tensor_copy(rhsB[:, :SL, O], axh[:, :SL])
                    for jj in range(K):
                        gt = gts[jj]
                        j8 = jj * 8
                        nc.vector.tensor_mul(
                            rhsA[:, j8:j8 + 8, :O], gt[:, :, :O],
                            axl[:, j8:j8 + 8, None].to_broadcast([128, 8, O]))
                        nc.vector.tensor_mul(
                            rhsB[:, j8:j8 + 8, :O], gt[:, :, TROW:TROW + O],
                            axh[:, j8:j8 + 8, None].to_broadcast([128, 8, O]))
                        for (o, n, wdx, b0, fi, la) in runs_by_chunk[ch0 + jj]:
                            oo = jj * 8 + o
                            c0 = (b0 - wdx * WIN) * (O + 1)
                            cn = n * (O + 1)
                            nc.tensor.matmul(
                                out=psacc[:, wdx, c0:c0 + cn],
                                lhsT=idf[:],
                                rhs=rhsA[:, oo:oo + n, :],
                                start=fi, stop=False,
                                skip_group_check=True)
                            nc.tensor.matmul(
                                out=psacc[:, wdx, c0:c0 + cn],
                                lhsT=idf[:],
                                rhs=rhsB[:, oo:oo + n, :],
                                start=False, stop=la,
                                skip_group_check=True)

                # ---- final: out = elu(S / (alpha_sum + 1e-8)) ----
                sacc = fp.tile([128, NB, O + 1], F32, tag="sacc")
                for wdx in range(WIN):
                    nc.scalar.activation(
                        sacc[:, wdx * WIN:(wdx + 1) * WIN, :].rearrange(
                            "p b c -> p (b c)"),
                        psacc[:, wdx, :WIN * (O + 1)], ACTF.Copy)

              rc = fp.tile([128, NB], F32, tag="rc")
              nc.vector.tensor_scalar(out=rc[:], in0=sacc[:, :, O],
                                      scalar1=1e-8, scalar2=None,
                                      op0=ALU.add)
              nc.vector.reciprocal(rc[:], rc[:])
              ov = fp.tile([128, NB, O], F32, tag="ov")
              nc.vector.tensor_mul(ov[:], sacc[:, :, :O],
                                   rc[:, :, None].to_broadcast([128, NB, O]))
              neg = sacc[:, :, :O]
              nc.vector.tensor_scalar(out=neg, in0=ov[:], scalar1=0.0,
                                      scalar2=None, op0=ALU.min)
              nc.scalar.activation(neg, neg, ACTF.Exp)
              nc.vector.tensor_scalar(out=ov[:], in0=ov[:], scalar1=0.0,
                                      scalar2=-1.0, op0=ALU.max,
                                      op1=ALU.add)
              nc.vector.tensor_add(ov[:], ov[:], neg)
              nc.sync.dma_start(
                  out_t[:].rearrange("(b p) f -> p b f", p=128), ov[:])

    nc.compile()
    _BUILD_CACHE[key] = nc
    return nc


def _in_maps(x, W, a, per_core, perms, sched):
    x = np.asarray(x, dtype=np.float32)
    W = np.ascontiguousarray(np.asarray(W, dtype=np.float32))
    a = np.ascontiguousarray(np.asarray(a, dtype=np.float32))
    N, F = x.shape
    NL = N // C
    xT16 = np.ascontiguousarray(x.T.astype(np.float16))
    NLP = sched["NLP"]
    in_maps = []
    for c in range(C):
        pc = per_core[c]
        xperm = np.zeros((F, NLP), np.float16)
        xperm[:, :NL] = xT16[:, c * NL + perms[c]]
        in_maps.append({
            "x16": xT16, "W": W, "a": a, "xperm": xperm,
            "sidx": pc["sidx"], "par": pc["par"], "wt": pc["wt"],
            "msk": pc["msk"],
        })
    return in_maps


def kernel(x, edge_index, edge_weight, W, a):
    x = np.asarray(x, dtype=np.float32)
    W = np.ascontiguousarray(np.asarray(W, dtype=np.float32))
    a = np.ascontiguousarray(np.asarray(a, dtype=np.float32))
    N, F = x.shape
    O = W.shape[1]
    NL = N // C

    sched, per_core, perms = _prep(edge_index, edge_weight, N)
    nc = _build(N, F, O, sched)

    in_maps = _in_maps(x, W, a, per_core, perms, sched)
    res = bass_utils.run_bass_kernel_spmd(nc, in_maps, core_ids=list(range(C)))

    out = np.empty((N, O), np.float32)
    for c in range(C):
        op = res.results[c]["out"]
        out[c * NL + perms[c]] = op[:NL]
    return out
